# revision 1
# baseline (speedup 1.0000x reference)
"""AttnBlock (GroupNorm + spatial self-attention + residual) on 8 trn2 NeuronCores.

Sharding: 8 cores = 2 batches x 4 query-chunks of 1024 spatial positions.
Each core receives x[b] rolled so its query range is columns [0, 1024); all
cores run one identical SPMD program.

Host-side algebra (exact up to dropped softmax-invariant terms):
  scores^T[j,i] = hn[:,j] . (Wqk hn[:,i] + bqk)   with Wqk = C^-1/2 wk^T wq,
    bqk = C^-1/2 wk^T bq  (the bk term is constant over j -> softmax-invariant)
  out = x + sum_j softmax_j . (Wov hn[:,j]) + bov  with Wov = wo wv,
    bov = wo bv + bo      (softmax rows sum to 1 -> bias moves outside)

Device-side GroupNorm folding: hn = A.x + B per channel. The B-term of the
keys side is softmax-invariant; the A-scale folds into the projection weight
rows, and residual B-terms fold into runtime-adjusted biases. So the PE
reads raw x everywhere and no normalized copy is ever materialized.

x arrives host-pre-blocked [NT, NS, 128, 512] so every DMA block is a
contiguous DRAM span (fewer descriptors -> faster issue).
"""

import ml_dtypes
import numpy as np

import concourse.bass as bass
import concourse.tile as tile
from concourse import bacc, mybir
from concourse import bass_utils

F32 = mybir.dt.float32
F32R = mybir.dt.float32r
BF16 = mybir.dt.bfloat16

B, C, D, H, W = 2, 512, 4, 32, 32
L = D * H * W            # 4096
G = 32                   # groupnorm groups
EPS = 1e-6
P = 128
NT = C // P              # 4 channel tiles
NJ = L // P              # 32 key tiles
NS = L // 512            # 512-col blocks per channel chunk
IC = 512                 # i-chunk width
LQ = 1024                # query cols per core
NIC = LQ // IC           # 2 i-chunks
NCORES = 8
DEPTH = 3                # attention software-pipeline depth (S^T groups ahead)

_CACHE = {}


def _build():
    nc = bacc.Bacc(trn_type="TRN2", target_bir_lowering=False, debug=False,
                   num_devices=NCORES)
    x_d = nc.dram_tensor("x", [NT, 2, P, L // 2], BF16, kind="ExternalInput").ap()
    xf_d = nc.dram_tensor("xf", [NT, P, LQ], F32R, kind="ExternalInput").ap()
    wqk_d = nc.dram_tensor("wqkT", [C, C], F32R, kind="ExternalInput").ap()
    wov_d = nc.dram_tensor("wovT", [C, C], BF16, kind="ExternalInput").ap()
    pg_d = nc.dram_tensor("pg", [C, G], F32R, kind="ExternalInput").ap()
    sel_d = nc.dram_tensor("sel", [G, C], F32R, kind="ExternalInput").ap()
    ones_d = nc.dram_tensor("ones", [P, P], BF16, kind="ExternalInput").ap()
    onesr_d = nc.dram_tensor("onesr", [1, P], F32R, kind="ExternalInput").ap()
    gamma_d = nc.dram_tensor("gamma", [C], F32, kind="ExternalInput").ap()
    wg_d = nc.dram_tensor("wgT", [G, C], F32R, kind="ExternalInput").ap()
    vg_d = nc.dram_tensor("vgT", [G, C], F32R, kind="ExternalInput").ap()
    hqk_d = nc.dram_tensor("hqk", [C], F32, kind="ExternalInput").ap()
    hov_d = nc.dram_tensor("hov", [C], F32, kind="ExternalInput").ap()
    out_d = nc.dram_tensor("out", [C, LQ], F32, kind="ExternalOutput").ap()

    with tile.TileContext(nc) as tc:
        with (
            tc.tile_pool(name="big", bufs=1) as big,
            tc.tile_pool(name="wp", bufs=1) as wp,
            tc.tile_pool(name="small", bufs=1) as small,
            tc.tile_pool(name="est", bufs=DEPTH + 5) as est,
            tc.tile_pool(name="osb", bufs=3) as osb,
            tc.tile_pool(name="zp", bufs=6) as zp,
            tc.tile_pool(name="tmp", bufs=4) as tmp,
            tc.tile_pool(name="ps", bufs=3, space="PSUM") as ps,
            tc.tile_pool(name="pho", bufs=4, space="PSUM") as pho,
            tc.tile_pool(name="psum1", bufs=1, space="PSUM") as psum1,
        ):
            # ---- DMA strategy: one large contiguous DMA per tensor (a single
            # HWDGE instruction parallelizes internally at ~280GB/s; multiple
            # DMAs on one queue serialize). x chunks go on the sync queue so
            # they arrive pipelined for stats; everything else streams in
            # parallel on the gpsimd (SWDGE) queue. ----
            xt = big.tile([P, NT, L], BF16, tag="xt")
            for t in (0, 2, 1, 3):
                for h in range(2):
                    nc.sync.dma_start(xt[:, t, bass.ts(h, L // 2)], x_d[t, h])
            pg = small.tile([P, NT, G], F32R, tag="pg")
            nc.gpsimd.dma_start(pg[:], pg_d.rearrange("(t p) g -> p t g", p=P))
            sel = small.tile([G, NT, P], F32R, tag="sel")
            nc.gpsimd.dma_start(sel[:], sel_d.rearrange("g (t p) -> g t p", p=P))
            ones = small.tile([P, P], BF16, tag="ones")
            nc.gpsimd.dma_start(ones[:], ones_d)
            onesr = small.tile([1, P], F32R, tag="onesr")
            nc.gpsimd.dma_start(onesr[:], onesr_d)
            gam = small.tile([P, NT], F32, tag="gam")
            nc.gpsimd.dma_start(gam[:], gamma_d.rearrange("(t p) -> p t", p=P))
            wg = small.tile([G, NT, P], F32R, tag="wg")
            nc.gpsimd.dma_start(wg[:], wg_d.rearrange("g (t p) -> g t p", p=P))
            vg = small.tile([G, NT, P], F32R, tag="vg")
            nc.gpsimd.dma_start(vg[:], vg_d.rearrange("g (t p) -> g t p", p=P))
            hqk = small.tile([P, NT], F32, tag="hqk")
            nc.gpsimd.dma_start(hqk[:], hqk_d.rearrange("(t p) -> p t", p=P))
            hov = small.tile([P, NT], F32, tag="hov")
            nc.gpsimd.dma_start(hov[:], hov_d.rearrange("(t p) -> p t", p=P))
            wqk = wp.tile([P, NT, C], F32R, tag="wqk")
            nc.sync.dma_start(wqk[:], wqk_d.rearrange("(t p) c -> p t c", p=P))
            xf = big.tile([P, NT, LQ], F32R, tag="xf")
            xf_r = xf_d.rearrange("t p l -> p t l")
            for icn in range(NIC):
                nc.sync.dma_start(xf[:, :, bass.ts(icn, IC)],
                                  xf_r[:, :, bass.ts(icn, IC)])
            wov = wp.tile([P, NT, C], BF16, tag="wov")
            nc.scalar.dma_start(wov[:], wov_d.rearrange("(t p) c -> p t c", p=P))

            # preload the Sqrt table set while DMA streams in
            epst = small.tile([G, 1], F32, tag="eps")
            nc.vector.memset(epst[:], EPS)
            dum = tmp.tile([G, 1], F32, tag="dum")
            nc.scalar.activation(dum[:], epst[:], mybir.ActivationFunctionType.Sqrt)

            # ---- groupnorm stats: ACT accum_out for t=0 (lands first), DVE
            # bn_stats for t=1..3
            # (ACT Copy/Square passes with the free affine scale give mean and
            # E[x^2] directly; splitting engines shortens the serial chain) ----
            m2 = small.tile([P, NT, 2], F32R, tag="m2")
            ajunk = tmp.tile([P, L], BF16, tag="ajunk")
            for t in range(NT):
                if t in (0, 1):
                    macc = tmp.tile([P, 2], F32, tag="macc")
                    nc.scalar.activation(ajunk[:], xt[:, t, :],
                                         mybir.ActivationFunctionType.Copy,
                                         scale=1.0 / L, accum_out=macc[:, 0:1])
                    nc.scalar.activation(ajunk[:], xt[:, t, :],
                                         mybir.ActivationFunctionType.Square,
                                         scale=1.0 / (L ** 0.5),
                                         accum_out=macc[:, 1:2])
                    nc.vector.tensor_copy(m2[:, t, :], macc[:])
                    continue
                st = tmp.tile([P, NS, 6], F32, tag="bnst")
                for s in range(NS):
                    nc.vector.bn_stats(st[:, s, :], xt[:, t, bass.ts(s, 512)])
                mv = tmp.tile([P, 2], F32, tag="bnmv")
                nc.vector.bn_aggr(mv[:], st[:])
                # m2 = [mean, var + mean^2] = [mean, E[x^2]]
                msq = tmp.tile([P, 1], F32, tag="msq")
                nc.vector.tensor_mul(msq[:], mv[:, 0:1], mv[:, 0:1])
                nc.vector.tensor_copy(m2[:, t, 0:1], mv[:, 0:1])
                nc.vector.tensor_add(m2[:, t, 1:2], mv[:, 1:2], msq[:])
            gps = ps.tile([G, 2], F32, tag="mm")
            for t in range(NT):
                nc.tensor.matmul(gps[:], pg[:, t, :], m2[:, t, :],
                                 start=(t == 0), stop=(t == NT - 1))
            # group stats -> [mean_g, rstd_g]
            gsb = small.tile([G, 2], F32R, tag="gsb")
            nc.vector.tensor_copy(gsb[:, 0:1], gps[:, 0:1])
            vrg = tmp.tile([G, 1], F32, tag="vrg")
            nc.vector.tensor_mul(vrg[:], gsb[:, 0:1].bitcast(F32), gsb[:, 0:1].bitcast(F32))
            nc.vector.tensor_tensor(vrg[:], gps[:, 1:2], vrg[:], mybir.AluOpType.subtract)
            nc.scalar.activation(vrg[:], vrg[:], mybir.ActivationFunctionType.Sqrt,
                                 bias=epst[:], scale=1.0)
            with nc.allow_low_precision(reason="fp32r rounding of rstd is ~1e-4"):
                nc.vector.reciprocal(gsb[:, 1:2], vrg[:])
            # broadcast to channels: chsb[p, t, 0:2] = [mean, rstd] per channel
            # (4 single-MM groups share one PSUM tile; finished groups only
            # lose has_written bits, not data)
            chsb = small.tile([P, NT, 2], F32, tag="chsb")
            chs = ps.tile([P, 2 * NT], F32, tag="mm")
            for t in range(NT):
                nc.tensor.matmul(chs[:, 2 * t:2 * t + 2], sel[:, t, :], gsb[:],
                                 start=True, stop=True)
            nc.vector.tensor_copy(chsb[:], chs[:])
            # A = rstd*gamma per channel
            A = small.tile([P, NT], F32, tag="A")
            nc.vector.tensor_mul(A[:], chsb[:, :, 1], gam[:])

            # ---- scale weight rows by A in place (TT w/ free-dim-0 broadcast:
            # tensor_scalar with AP scalars has a ~3.3us ucode overhead) ----
            for tq in range(NT):
                nc.vector.tensor_tensor(wqk[:, :, bass.ts(tq, P)],
                                        wqk[:, :, bass.ts(tq, P)],
                                        A[:, :, None].to_broadcast((P, NT, P)),
                                        mybir.AluOpType.mult)
            for t in range(NT):
                eng = nc.vector if t < 2 else nc.gpsimd
                eng.tensor_tensor(wov[:, t, :], wov[:, t, :],
                                  A[:, t:t + 1].to_broadcast((P, C)),
                                  mybir.AluOpType.mult)

            # ---- qk_eff[:, i] = A . (WqkA x_sl + bqkE) over query cols [0, LQ) ----
            qk = big.tile([P, NT, LQ], BF16, tag="qk")
            emitted_bias = [False]

            bias_tiles = {}

            def emit_bias():
                # s_g = mean_g * rstd_g (padded to 2 cols for fp32r matmul rhs)
                st2 = small.tile([G, 2], F32R, tag="st2")
                nc.vector.tensor_mul(st2[:, 0:1], gsb[:, 0:1].bitcast(F32), gsb[:, 1:2].bitcast(F32))
                nc.vector.tensor_copy(st2[:, 1:2], gsb[:, 0:1].bitcast(F32))
                # bias folds: bqkE = hqk - Wg.s, bovE = hov - Vg.s (host-folded
                # per-group partial matvecs; 8 tiny K=32 MMs off the critical chain)
                bqkE = small.tile([P, NT], F32, tag="bqkE")
                bovE = small.tile([P, NT], F32, tag="bovE")
                psB = ps.tile([P, 4 * NT], F32, tag="mm")
                for tq in range(NT):
                    nc.tensor.matmul(psB[:, 2 * tq:2 * tq + 2], wg[:, tq, :], st2[:],
                                         start=True, stop=True)
                    nc.tensor.matmul(psB[:, 2 * NT + 2 * tq:2 * NT + 2 * tq + 2],
                                         vg[:, tq, :], st2[:], start=True, stop=True)
                psBv = psB.rearrange("p (c two) -> p c two", two=2)
                nc.vector.tensor_tensor(bqkE[:], hqk[:], psBv[:, 0:NT, 0],
                                                mybir.AluOpType.subtract)
                nc.vector.tensor_tensor(bovE[:], hov[:], psBv[:, NT:2 * NT, 0],
                                                mybir.AluOpType.subtract)
                bias_tiles["bqkE"] = bqkE
                bias_tiles["bovE"] = bovE
                emitted_bias[0] = True

            for icn in range(NIC):
                for tq in range(NT):
                    qps = ps.tile([P, IC], F32, tag="mm")
                    for t in range(NT):
                        nc.tensor.matmul(qps[:], wqk[:, t, bass.ts(tq, P)],
                                         xf[:, t, bass.ts(icn, IC)],
                                         start=(t == 0), stop=(t == NT - 1))
                    if not emitted_bias[0]:
                        emit_bias()
                    qtmp = tmp.tile([P, IC], F32, tag="qtmp")
                    nc.vector.tensor_tensor(qtmp[:], qps[:],
                                            bias_tiles["bqkE"][:, tq:tq + 1].to_broadcast((P, IC)),
                                            mybir.AluOpType.add)
                    nc.vector.tensor_tensor(qk[:, tq, bass.ts(icn, IC)], qtmp[:],
                                            A[:, tq:tq + 1].to_broadcast((P, IC)),
                                            mybir.AluOpType.mult)

            if not emitted_bias[0]:
                emit_bias()

            # ---- z[t][icn] = x_residual + bovE on GpSimd (idle here) ----
            zall = {}
            for icn in range(NIC):
                for t in range(NT):
                    z = zp.tile([P, IC], F32, tag="zp", name=f"z{icn}_{t}")
                    nc.gpsimd.tensor_tensor(z[:], xf[:, t, bass.ts(icn, IC)].bitcast(F32),
                                            bias_tiles["bovE"][:, t:t + 1].to_broadcast((P, IC)),
                                            mybir.AluOpType.add)
                    zall[(icn, t)] = z

            # ---- voT projection: voT[j, c] = (WovA x)[c, j]^T ----
            vot = big.tile([P, NJ, C], BF16, tag="vot")
            for j in range(NJ):
                vps = ps.tile([P, C], F32, tag="mm")
                for t in range(NT):
                    nc.tensor.matmul(vps[:], xt[:, t, bass.ts(j, P)], wov[:, t, :],
                                     start=(t == 0), stop=(t == NT - 1))
                nc.scalar.copy(vot[:, j, :], vps[:])

            # ---- attention per i-chunk ----
            pending_fin = [None]

            def make_finalize(icn, sums, hops):
                def fin():
                    zs = [zall[(icn, t)] for t in range(NT)]
                    rec = small.tile([1, IC], F32R, tag=f"rec{icn}",
                                     name=f"rec{icn}")
                    with nc.allow_low_precision(reason="fp32r denom ~1e-4"):
                        nc.vector.reciprocal(rec[:], sums[0:1, :])
                    rps = ps.tile([P, IC], F32, tag="mm", name=f"rps{icn}")
                    nc.tensor.matmul(rps[:], onesr[:], rec[:], start=True, stop=True)
                    rbc = tmp.tile([P, IC], F32, tag="rbc", name=f"rbc{icn}")
                    nc.vector.tensor_copy(rbc[:], rps[:])
                    last = icn == NIC - 1
                    for t in range(NT):
                        o = osb.tile([P, IC], F32, tag="osb", name=f"o{icn}_{t}")
                        nc.vector.tensor_tensor(o[:], hops[t][:], rbc[:],
                                                mybir.AluOpType.mult)
                        eng = nc.gpsimd if (last and t >= 2) else nc.vector
                        eng.tensor_tensor(o[:], o[:], zs[t][:],
                                          mybir.AluOpType.add)
                        nc.sync.dma_start(out_d[bass.ts(t, P), bass.ts(icn, IC)], o[:])
                return fin

            for icn in range(NIC):
                sums = psum1.tile([P, IC], F32, tag="sums", name=f"sums{icn}")
                hops = [pho.tile([P, IC], F32, tag="ho", name=f"ho_{icn}_{t}")
                        for t in range(NT)]
                ests = [None] * NJ

                def consume(jj, sums=sums, hops=hops, ests=ests):
                    es = ests[jj]
                    nc.tensor.matmul(sums[:], ones[:], es[:],
                                     start=(jj == 0), stop=(jj == NJ - 1))
                    for t in range(NT):
                        nc.tensor.matmul(hops[t][:], vot[:, jj, bass.ts(t, P)],
                                         es[:],
                                         start=(jj == 0), stop=(jj == NJ - 1))
                    ests[jj] = None

                for j in range(NJ):
                    if j == 2 and pending_fin[0] is not None:
                        pending_fin[0]()
                        pending_fin[0] = None
                    sps = ps.tile([P, IC], F32, tag="mm", name=f"sps{icn}_{j}")
                    for t in range(NT):
                        nc.tensor.matmul(sps[:], xt[:, t, bass.ts(j, P)],
                                         qk[:, t, bass.ts(icn, IC)],
                                         start=(t == 0), stop=(t == NT - 1))
                    es = est.tile([P, IC], BF16, tag="est", name=f"est{icn}_{j}")
                    nc.scalar.activation(es[:], sps[:],
                                         mybir.ActivationFunctionType.Exp)
                    ests[j] = es
                    if j >= DEPTH:
                        consume(j - DEPTH)
                for j in range(NJ - DEPTH, NJ):
                    consume(j)
                pending_fin[0] = make_finalize(icn, sums, hops)
            pending_fin[0]()

    nc.compile()
    return nc


def _prep(inputs):
    s = float(C) ** -0.5
    wq = np.asarray(inputs["wq"], np.float64)
    wk = np.asarray(inputs["wk"], np.float64)
    wv = np.asarray(inputs["wv"], np.float64)
    wo = np.asarray(inputs["wo"], np.float64)
    bq = np.asarray(inputs["bq"], np.float64)
    bv = np.asarray(inputs["bv"], np.float64)
    bo = np.asarray(inputs["bo"], np.float64)
    gamma = np.asarray(inputs["gamma"], np.float64)
    beta = np.asarray(inputs["beta"], np.float64)
    Wqk = (wk.T @ wq).T * s      # lhsT layout [c_in, c_out]; Wqk.T is [c_out, c_in]
    Wov = (wo @ wv).T            # [c_in, c_out]
    bqkv = (wk.T @ bq) * s
    bovv = wo @ bv + bo
    # per-group partial matvecs with gamma folded: WgT[g, c] = sum_{c' in g} Wqk[c,c'] gamma[c']
    GS = C // G
    WgT = (Wqk * gamma[:, None]).reshape(G, GS, C).sum(axis=1)
    VgT = (Wov * gamma[:, None]).reshape(G, GS, C).sum(axis=1)
    consts = {
        "wqkT": np.ascontiguousarray(Wqk, np.float32),
        "wovT": np.ascontiguousarray(Wov).astype(ml_dtypes.bfloat16),
        "wgT": np.ascontiguousarray(WgT, np.float32),
        "vgT": np.ascontiguousarray(VgT, np.float32),
        "hqk": (Wqk.T @ beta + bqkv).astype(np.float32),
        "hov": (Wov.T @ beta + bovv).astype(np.float32),
        "gamma": np.asarray(inputs["gamma"], np.float32),
        "pg": np.ascontiguousarray(
            (np.arange(C)[:, None] // (C // G) == np.arange(G)[None, :])
            .astype(np.float32) / (C // G)),
        "sel": np.ascontiguousarray(
            (np.arange(G)[:, None] == np.arange(C)[None, :] // (C // G))
            .astype(np.float32)),
        "ones": np.ones((P, P), ml_dtypes.bfloat16),
        "onesr": np.ones((1, P), np.float32),
    }
    return consts


LAST_RESULTS = None


def kernel(**inputs) -> np.ndarray:
    global LAST_RESULTS
    if "nc" not in _CACHE:
        _CACHE["nc"] = _build()
    nc = _CACHE["nc"]
    consts = _prep(inputs)
    x = np.asarray(inputs["x"], np.float32)
    xb = x.reshape(B, C, L)
    in_maps = []
    for core in range(NCORES):
        b, chunk = divmod(core, 4)
        xr = np.roll(xb[b], -LQ * chunk, axis=1)
        xblk = np.ascontiguousarray(
            xr.reshape(NT, P, 2, L // 2).swapaxes(1, 2)).astype(ml_dtypes.bfloat16)
        xf = np.ascontiguousarray(xr[:, :LQ].reshape(NT, P, LQ))
        in_maps.append({"x": xblk, "xf": xf, **consts})
    res = bass_utils.run_bass_kernel_spmd(nc, in_maps, core_ids=list(range(NCORES)))
    LAST_RESULTS = res
    out = np.empty((B, C, L), np.float32)
    for core in range(NCORES):
        b, chunk = divmod(core, 4)
        out[b][:, LQ * chunk:LQ * (chunk + 1)] = res.results[core]["out"]
    return out.reshape(B, C, D, H, W)



# revision 5
# speedup vs baseline: 1.4320x; 1.4320x over previous
"""AttnBlock (GroupNorm + spatial self-attention + residual) on 8 trn2 NeuronCores.

Sharding: 8 cores = 2 batches x 4 query-chunks of 1024 spatial positions.
Each core receives x[b] rolled so its query range is columns [0, 1024); all
cores run one identical SPMD program.

Host-side algebra (exact up to dropped softmax-invariant terms):
  scores^T[j,i] = hn[:,j] . (Wqk hn[:,i] + bqk)   with Wqk = C^-1/2 wk^T wq,
    bqk = C^-1/2 wk^T bq  (the bk term is constant over j -> softmax-invariant)
  out = x + sum_j softmax_j . (Wov hn[:,j]) + bov  with Wov = wo wv,
    bov = wo bv + bo      (softmax rows sum to 1 -> bias moves outside)

Device-side GroupNorm folding: hn = A.x + B per channel. The B-term of the
keys side is softmax-invariant; the A-scale folds into the projection weight
rows, and residual B-terms fold into runtime-adjusted biases. So the PE
reads raw x everywhere and no normalized copy is ever materialized.

fp8 plan: all five big matmul groups run fp8e4m3 with DoubleRow perf mode
(2 MACs/PE/cycle): qk projection (weights x256 host-prescaled), voT
projection (weights x8), scores, ho accumulation, and softmax sums. The
softmax shift exp(s-5) keeps es in fp8 range (max logit ~9.2); the shift is
softmax-invariant. Residual path stays f32 via a separate xf copy. rstd is
computed as exp(-0.5*ln(var+eps)) so every ACT func lives in the single
natural_log_exp table set (no mid-kernel table reloads).
"""

import ml_dtypes
import numpy as np

import concourse.bass as bass
import concourse.tile as tile
from concourse import bacc, mybir
from concourse import bass_utils

F32 = mybir.dt.float32
F32R = mybir.dt.float32r
BF16 = mybir.dt.bfloat16
FP8 = mybir.dt.float8e4
DR = mybir.MatmulPerfMode.DoubleRow

NPF8 = ml_dtypes.float8_e4m3fn

B, C, D, H, W = 2, 512, 4, 32, 32
L = D * H * W            # 4096
G = 32                   # groupnorm groups
EPS = 1e-6
P = 128
NT = C // P              # 4 channel tiles
NJ = L // P              # 32 key tiles
NS = L // 512            # 512-col blocks per channel chunk
IC = 512                 # i-chunk width
LQ = 1024                # query cols per core
NIC = LQ // IC           # 2 i-chunks
NCORES = 8
NPAIR = NJ // 2          # 16 key-tile pairs per i-chunk
PD = 2                   # attention software-pipeline depth (es pairs ahead)

QK_SCALE = 256.0         # host prescale on Wqk (fp8 subnormal avoidance)
OV_SCALE = 8.0           # host prescale on Wov
ES_SHIFT = -5.0          # softmax-invariant logit shift (max logit ~9.2)

_CACHE = {}


def _build():
    nc = bacc.Bacc(trn_type="TRN2", target_bir_lowering=False, debug=False,
                   num_devices=NCORES)
    x_d = nc.dram_tensor("x8", [NT, 2, P, L // 2], FP8, kind="ExternalInput").ap()
    xf_d = nc.dram_tensor("xf", [NT, P, LQ], F32, kind="ExternalInput").ap()
    wqk_d = nc.dram_tensor("wqkT", [C, C], BF16, kind="ExternalInput").ap()
    wov_d = nc.dram_tensor("wovT", [C, C], BF16, kind="ExternalInput").ap()
    pg_d = nc.dram_tensor("pg", [C, G], F32R, kind="ExternalInput").ap()
    sel_d = nc.dram_tensor("sel", [G, C], F32R, kind="ExternalInput").ap()
    ones8_d = nc.dram_tensor("ones8", [P, 2 * P], FP8, kind="ExternalInput").ap()
    onesr_d = nc.dram_tensor("onesr", [1, P], F32R, kind="ExternalInput").ap()
    gamma_d = nc.dram_tensor("gamma", [C], F32, kind="ExternalInput").ap()
    wg_d = nc.dram_tensor("wgT", [G, C], F32R, kind="ExternalInput").ap()
    vg_d = nc.dram_tensor("vgT", [G, C], F32R, kind="ExternalInput").ap()
    hqk_d = nc.dram_tensor("hqk", [C], F32, kind="ExternalInput").ap()
    hov_d = nc.dram_tensor("hov", [C], F32, kind="ExternalInput").ap()
    out_d = nc.dram_tensor("out", [C, LQ], F32, kind="ExternalOutput").ap()

    with tile.TileContext(nc) as tc:
        with (
            tc.tile_pool(name="big", bufs=1) as big,
            tc.tile_pool(name="wp", bufs=1) as wp,
            tc.tile_pool(name="small", bufs=1) as small,
            tc.tile_pool(name="est", bufs=PD + 3) as est,
            tc.tile_pool(name="osb", bufs=3) as osb,
            tc.tile_pool(name="zp", bufs=8) as zp,
            tc.tile_pool(name="tmp", bufs=4) as tmp,
            tc.tile_pool(name="ps", bufs=3, space="PSUM") as ps,
            tc.tile_pool(name="pho", bufs=4, space="PSUM") as pho,
            tc.tile_pool(name="psum1", bufs=1, space="PSUM") as psum1,
        ):
            # ---- DMA: x8 split across the two HWDGE rings (sync: t0,t1 for
            # ACT stats; scalar: t2,t3 for DVE stats + weights after). ----
            xt8 = big.tile([P, NT, L], FP8, tag="xt8")
            for t in (0, 1):
                for h in range(2):
                    nc.sync.dma_start(xt8[:, t, bass.ts(h, L // 2)], x_d[t, h])
            for t in (2, 3):
                for h in range(2):
                    nc.scalar.dma_start(xt8[:, t, bass.ts(h, L // 2)], x_d[t, h])
            wqk = wp.tile([P, NT, C], BF16, tag="wqk")
            nc.scalar.dma_start(wqk[:], wqk_d.rearrange("(t p) c -> p t c", p=P))
            wov = wp.tile([P, NT, C], BF16, tag="wov")
            nc.scalar.dma_start(wov[:], wov_d.rearrange("(t p) c -> p t c", p=P))
            xf = big.tile([P, NT, LQ], F32, tag="xf")
            xf_r = xf_d.rearrange("t p l -> p t l")
            for icn in range(NIC):
                nc.sync.dma_start(xf[:, :, bass.ts(icn, IC)],
                                  xf_r[:, :, bass.ts(icn, IC)])
            pg = small.tile([P, NT, G], F32R, tag="pg")
            nc.gpsimd.dma_start(pg[:], pg_d.rearrange("(t p) g -> p t g", p=P))
            sel = small.tile([G, NT, P], F32R, tag="sel")
            nc.gpsimd.dma_start(sel[:], sel_d.rearrange("g (t p) -> g t p", p=P))
            ones8 = small.tile([P, 2, P], FP8, tag="ones8")
            nc.gpsimd.dma_start(ones8[:], ones8_d)
            onesr = small.tile([1, P], F32R, tag="onesr")
            nc.gpsimd.dma_start(onesr[:], onesr_d)
            gam = small.tile([P, NT], F32, tag="gam")
            nc.gpsimd.dma_start(gam[:], gamma_d.rearrange("(t p) -> p t", p=P))
            wg = small.tile([G, NT, P], F32R, tag="wg")
            nc.gpsimd.dma_start(wg[:], wg_d.rearrange("g (t p) -> g t p", p=P))
            vg = small.tile([G, NT, P], F32R, tag="vg")
            nc.gpsimd.dma_start(vg[:], vg_d.rearrange("g (t p) -> g t p", p=P))
            hqk = small.tile([P, NT], F32, tag="hqk")
            nc.gpsimd.dma_start(hqk[:], hqk_d.rearrange("(t p) -> p t", p=P))
            hov = small.tile([P, NT], F32, tag="hov")
            nc.gpsimd.dma_start(hov[:], hov_d.rearrange("(t p) -> p t", p=P))

            # preload the natural_log_exp table set while DMA streams in
            # (Ln/Exp/Copy/Identity/Square all live in this one set)
            epst = small.tile([G, 1], F32, tag="eps")
            nc.vector.memset(epst[:], EPS)
            zg = small.tile([G, 1], F32, tag="zg")
            nc.vector.memset(zg[:], 0.0)
            shf = small.tile([P, 1], F32, tag="shf")
            nc.vector.memset(shf[:], ES_SHIFT)
            dum = tmp.tile([G, 1], F32, tag="dum")
            nc.scalar.activation(dum[:], epst[:], mybir.ActivationFunctionType.Ln,
                                 bias=zg[:])

            # ---- groupnorm stats from fp8 x: ACT accum for t=0, DVE
            # bn_stats for t=1..3 ----
            m2 = small.tile([P, NT, 2], F32R, tag="m2")
            ajunk = tmp.tile([P, L], BF16, tag="ajunk")
            for t in range(NT):
                if t == 0:
                    macc = tmp.tile([P, 2], F32, tag="macc")
                    nc.scalar.activation(ajunk[:], xt8[:, t, :],
                                         mybir.ActivationFunctionType.Copy,
                                         scale=1.0 / L, accum_out=macc[:, 0:1])
                    nc.scalar.activation(ajunk[:], xt8[:, t, :],
                                         mybir.ActivationFunctionType.Square,
                                         scale=1.0 / (L ** 0.5),
                                         accum_out=macc[:, 1:2])
                    nc.vector.tensor_copy(m2[:, t, :], macc[:])
                    continue
                st = tmp.tile([P, NS, 6], F32, tag="bnst")
                for s in range(NS):
                    nc.vector.bn_stats(st[:, s, :], xt8[:, t, bass.ts(s, 512)])
                mv = tmp.tile([P, 2], F32, tag="bnmv")
                nc.vector.bn_aggr(mv[:], st[:])
                # m2 = [mean, var + mean^2] = [mean, E[x^2]]
                msq = tmp.tile([P, 1], F32, tag="msq")
                nc.vector.tensor_mul(msq[:], mv[:, 0:1], mv[:, 0:1])
                nc.vector.tensor_copy(m2[:, t, 0:1], mv[:, 0:1])
                nc.vector.tensor_add(m2[:, t, 1:2], mv[:, 1:2], msq[:])
            gps = ps.tile([G, 2], F32, tag="mm")
            for t in range(NT):
                nc.tensor.matmul(gps[:], pg[:, t, :], m2[:, t, :],
                                 start=(t == 0), stop=(t == NT - 1))
            # group stats -> [mean_g, rstd_g]; rstd = exp(-0.5*ln(var+eps))
            gsb = small.tile([G, 2], F32R, tag="gsb")
            nc.vector.tensor_copy(gsb[:, 0:1], gps[:, 0:1])
            vrg = tmp.tile([G, 1], F32, tag="vrg")
            nc.vector.tensor_mul(vrg[:], gsb[:, 0:1].bitcast(F32), gsb[:, 0:1].bitcast(F32))
            nc.vector.tensor_tensor(vrg[:], gps[:, 1:2], vrg[:], mybir.AluOpType.subtract)
            lnv = tmp.tile([G, 1], F32, tag="lnv")
            nc.scalar.activation(lnv[:], vrg[:], mybir.ActivationFunctionType.Ln,
                                 bias=epst[:], scale=1.0)
            nc.scalar.activation(gsb[:, 1:2], lnv[:],
                                 mybir.ActivationFunctionType.Exp, scale=-0.5,
                                 bias=zg[:])
            # broadcast to channels: chsb[p, t, 0:2] = [mean, rstd] per channel
            chsb = small.tile([P, NT, 2], F32, tag="chsb")
            chs = ps.tile([P, 2 * NT], F32, tag="mm")
            for t in range(NT):
                nc.tensor.matmul(chs[:, 2 * t:2 * t + 2], sel[:, t, :], gsb[:],
                                 start=True, stop=True)
            nc.vector.tensor_copy(chsb[:], chs[:])
            # A = rstd*gamma per channel
            A = small.tile([P, NT], F32, tag="A")
            nc.vector.tensor_mul(A[:], chsb[:, :, 1], gam[:])
            As = small.tile([P, NT], F32, tag="As")
            nc.scalar.activation(As[:], A[:], mybir.ActivationFunctionType.Copy,
                                 scale=1.0 / QK_SCALE)

            # ---- bias folds: bqkE = hqk - Wg.s, bovE = hov - Vg.s ----
            st2 = small.tile([G, 2], F32R, tag="st2")
            nc.vector.tensor_mul(st2[:, 0:1], gsb[:, 0:1].bitcast(F32), gsb[:, 1:2].bitcast(F32))
            nc.vector.tensor_copy(st2[:, 1:2], gsb[:, 0:1].bitcast(F32))
            bqkE = small.tile([P, NT], F32, tag="bqkE")
            bovE = small.tile([P, NT], F32, tag="bovE")
            psB = ps.tile([P, 4 * NT], F32, tag="mm")
            for tq in range(NT):
                nc.tensor.matmul(psB[:, 2 * tq:2 * tq + 2], wg[:, tq, :], st2[:],
                                 start=True, stop=True)
                nc.tensor.matmul(psB[:, 2 * NT + 2 * tq:2 * NT + 2 * tq + 2],
                                 vg[:, tq, :], st2[:], start=True, stop=True)
            psBv = psB.rearrange("p (c two) -> p c two", two=2)
            nc.vector.tensor_tensor(bqkE[:], hqk[:], psBv[:, 0:NT, 0],
                                    mybir.AluOpType.subtract)
            nc.vector.tensor_tensor(bovE[:], hov[:], psBv[:, NT:2 * NT, 0],
                                    mybir.AluOpType.subtract)
            qkb = small.tile([P, NT], F32, tag="qkb")
            nc.vector.tensor_mul(qkb[:], A[:], bqkE[:])

            # ---- quantize weights with the A fold: wqk8 = fp8(wqk*A),
            # wov8 = fp8(wov*A) (host prescales QK_SCALE / OV_SCALE) ----
            wqk8 = wp.tile([P, NT, C], FP8, tag="wqk8")
            wov8 = wp.tile([P, NT, C], FP8, tag="wov8")
            for t in range(NT):
                nc.scalar.activation(wqk8[:, t, :], wqk[:, t, :],
                                     mybir.ActivationFunctionType.Copy,
                                     scale=A[:, t:t + 1])
                nc.scalar.activation(wov8[:, t, :], wov[:, t, :],
                                     mybir.ActivationFunctionType.Copy,
                                     scale=A[:, t:t + 1])

            # ---- qk8[:, i] = fp8(A.(WqkA x + bqkE)) over query cols [0, LQ)
            # (DoubleRow fp8: 2 c-tile pairs per psum) ----
            qk8 = big.tile([P, NT, LQ], FP8, tag="qk8")
            for icn in range(NIC):
                for tq in range(NT):
                    qps = ps.tile([P, IC], F32, tag="mm")
                    for u in range(2):
                        nc.tensor.matmul(qps[:], wqk8[:, 2 * u:2 * u + 2, bass.ts(tq, P)],
                                         xt8[:, 2 * u:2 * u + 2, bass.ts(icn, IC)],
                                         start=(u == 0), stop=(u == 1), perf_mode=DR)
                    nc.scalar.activation(qk8[:, tq, bass.ts(icn, IC)], qps[:],
                                         mybir.ActivationFunctionType.Identity,
                                         scale=As[:, tq:tq + 1],
                                         bias=qkb[:, tq:tq + 1])

            # ---- z[t][icn] = x_residual + bovE on GpSimd (idle here) ----
            zall = {}
            for icn in range(NIC):
                for t in range(NT):
                    z = zp.tile([P, IC], F32, tag="zp", name=f"z{icn}_{t}")
                    nc.gpsimd.tensor_tensor(z[:], xf[:, t, bass.ts(icn, IC)],
                                            bovE[:, t:t + 1].to_broadcast((P, IC)),
                                            mybir.AluOpType.add)
                    zall[(icn, t)] = z

            # ---- voT projection: voT[j, c] = fp8((WovA x)[c, j]^T) ----
            vot8 = big.tile([P, NJ, C], FP8, tag="vot8")
            for j in range(NJ):
                vps = ps.tile([P, C], F32, tag="mm")
                for u in range(2):
                    nc.tensor.matmul(vps[:], xt8[:, 2 * u:2 * u + 2, bass.ts(j, P)],
                                     wov8[:, 2 * u:2 * u + 2, :],
                                     start=(u == 0), stop=(u == 1), perf_mode=DR)
                nc.vector.tensor_copy(vot8[:, j, :], vps[:])

            # ---- attention per i-chunk ----
            pending_fin = [None]

            def make_finalize(icn, sums, hops):
                def fin():
                    zs = [zall[(icn, t)] for t in range(NT)]
                    rec = small.tile([1, IC], F32R, tag=f"rec{icn}",
                                     name=f"rec{icn}")
                    with nc.allow_low_precision(reason="fp32r denom ~1e-4"):
                        nc.vector.reciprocal(rec[:], sums[0:1, :])
                    rps = ps.tile([P, IC], F32, tag="mm", name=f"rps{icn}")
                    nc.tensor.matmul(rps[:], onesr[:], rec[:], start=True, stop=True)
                    rbc = tmp.tile([P, IC], F32, tag="rbc", name=f"rbc{icn}")
                    nc.vector.tensor_copy(rbc[:], rps[:])
                    last = icn == NIC - 1
                    for t in range(NT):
                        o = osb.tile([P, IC], F32, tag="osb", name=f"o{icn}_{t}")
                        nc.vector.tensor_tensor(o[:], hops[t][:], rbc[:],
                                                mybir.AluOpType.mult)
                        eng = nc.gpsimd if (last and t >= 2) else nc.vector
                        eng.tensor_tensor(o[:], o[:], zs[t][:],
                                          mybir.AluOpType.add)
                        nc.sync.dma_start(out_d[bass.ts(t, P), bass.ts(icn, IC)], o[:])
                return fin

            for icn in range(NIC):
                sums = psum1.tile([P, IC], F32, tag="sums", name=f"sums{icn}")
                hops = [pho.tile([P, IC], F32, tag="ho", name=f"ho_{icn}_{t}")
                        for t in range(NT)]
                espairs = [None] * NPAIR

                def consume(u, sums=sums, hops=hops, espairs=espairs):
                    es = espairs[u]
                    nc.tensor.matmul(sums[:], ones8[:, 0:2, :], es[:, 0:2, :],
                                     start=(u == 0), stop=(u == NPAIR - 1),
                                     perf_mode=DR)
                    for t in range(NT):
                        nc.tensor.matmul(hops[t][:],
                                         vot8[:, 2 * u:2 * u + 2, bass.ts(t, P)],
                                         es[:, 0:2, :],
                                         start=(u == 0), stop=(u == NPAIR - 1),
                                         perf_mode=DR)
                    espairs[u] = None

                escur = [None]
                for j in range(NJ):
                    u, par = divmod(j, 2)
                    if j == 2 and pending_fin[0] is not None:
                        pending_fin[0]()
                        pending_fin[0] = None
                    sps = ps.tile([P, IC], F32, tag="mm", name=f"sps{icn}_{j}")
                    for uu in range(2):
                        nc.tensor.matmul(sps[:], xt8[:, 2 * uu:2 * uu + 2, bass.ts(j, P)],
                                         qk8[:, 2 * uu:2 * uu + 2, bass.ts(icn, IC)],
                                         start=(uu == 0), stop=(uu == 1), perf_mode=DR)
                    if par == 0:
                        escur[0] = est.tile([P, 2, IC], FP8, tag="est",
                                            name=f"es{icn}_{u}")
                    nc.scalar.activation(escur[0][:, par, :], sps[:],
                                         mybir.ActivationFunctionType.Exp,
                                         bias=shf[:])
                    if par == 1:
                        espairs[u] = escur[0]
                        if u >= PD:
                            consume(u - PD)
                for u in range(NPAIR - PD, NPAIR):
                    consume(u)
                pending_fin[0] = make_finalize(icn, sums, hops)
            pending_fin[0]()

    nc.compile()
    return nc


def _prep(inputs):
    s = float(C) ** -0.5
    wq = np.asarray(inputs["wq"], np.float64)
    wk = np.asarray(inputs["wk"], np.float64)
    wv = np.asarray(inputs["wv"], np.float64)
    wo = np.asarray(inputs["wo"], np.float64)
    bq = np.asarray(inputs["bq"], np.float64)
    bv = np.asarray(inputs["bv"], np.float64)
    bo = np.asarray(inputs["bo"], np.float64)
    gamma = np.asarray(inputs["gamma"], np.float64)
    beta = np.asarray(inputs["beta"], np.float64)
    Wqk = (wk.T @ wq).T * s      # lhsT layout [c_in, c_out]
    Wov = (wo @ wv).T            # [c_in, c_out]
    bqkv = (wk.T @ bq) * s
    bovv = wo @ bv + bo
    # per-group partial matvecs with gamma folded
    GS = C // G
    WgT = (Wqk * gamma[:, None]).reshape(G, GS, C).sum(axis=1)
    VgT = (Wov * gamma[:, None]).reshape(G, GS, C).sum(axis=1)
    consts = {
        "wqkT": np.ascontiguousarray(Wqk * QK_SCALE).astype(ml_dtypes.bfloat16),
        "wovT": np.ascontiguousarray(Wov * OV_SCALE).astype(ml_dtypes.bfloat16),
        "wgT": np.ascontiguousarray(WgT, np.float32),
        "vgT": np.ascontiguousarray(VgT, np.float32),
        "hqk": (Wqk.T @ beta + bqkv).astype(np.float32),
        "hov": (Wov.T @ beta + bovv).astype(np.float32),
        "gamma": np.asarray(inputs["gamma"], np.float32),
        "pg": np.ascontiguousarray(
            (np.arange(C)[:, None] // (C // G) == np.arange(G)[None, :])
            .astype(np.float32) / (C // G)),
        "sel": np.ascontiguousarray(
            (np.arange(G)[:, None] == np.arange(C)[None, :] // (C // G))
            .astype(np.float32)),
        "ones8": np.ones((P, 2 * P), NPF8),
        "onesr": np.full((1, P), 1.0 / OV_SCALE, np.float32),
    }
    return consts


LAST_RESULTS = None


def kernel(**inputs) -> np.ndarray:
    global LAST_RESULTS
    if "nc" not in _CACHE:
        _CACHE["nc"] = _build()
    nc = _CACHE["nc"]
    consts = _prep(inputs)
    x = np.asarray(inputs["x"], np.float32)
    xb = x.reshape(B, C, L)
    in_maps = []
    for core in range(NCORES):
        b, chunk = divmod(core, 4)
        xr = np.roll(xb[b], -LQ * chunk, axis=1)
        x8 = np.ascontiguousarray(
            xr.reshape(NT, P, 2, L // 2).swapaxes(1, 2)).astype(NPF8)
        xf = np.ascontiguousarray(xr[:, :LQ].reshape(NT, P, LQ))
        in_maps.append({"x8": x8, "xf": xf, **consts})
    res = bass_utils.run_bass_kernel_spmd(nc, in_maps, core_ids=list(range(NCORES)))
    LAST_RESULTS = res
    out = np.empty((B, C, L), np.float32)
    for core in range(NCORES):
        b, chunk = divmod(core, 4)
        out[b][:, LQ * chunk:LQ * (chunk + 1)] = res.results[core]["out"]
    return out.reshape(B, C, D, H, W)


# revision 27
# speedup vs baseline: 1.6830x; 1.1752x over previous
"""AttnBlock (GroupNorm + spatial self-attention + residual) on 8 trn2 NeuronCores.

Sharding: 8 cores = 2 batches x 4 query-chunks of 1024 spatial positions.
Each core receives x[b] rolled so its query range is columns [0, 1024); all
cores run one identical SPMD program.

Host-side algebra (exact up to dropped softmax-invariant terms):
  scores^T[j,i] = hn[:,j] . (Wqk hn[:,i] + bqk)   with Wqk = C^-1/2 wk^T wq,
    bqk = C^-1/2 wk^T bq  (the bk term is constant over j -> softmax-invariant)
  out = x + sum_j softmax_j . (Wov hn[:,j]) + bov  with Wov = wo wv,
    bov = wo bv + bo      (softmax rows sum to 1 -> bias moves outside)

Device-side GroupNorm folding: hn = A.x + B per channel; the A-scales fold
into runtime-quantized fp8 weights, B-terms into runtime biases. All five
big matmul groups run fp8e4m3 DoubleRow (2 MACs/PE/cycle). exp(s-5) keeps
es in fp8 range (softmax-invariant shift). Residual stays f32 via xf.

Startup-latency engineering: partition-major DRAM layouts give 4-16KB DMA
packets; groupnorm stats are split per-tile between ACT (first half,
Copy/Square accum) and DVE (second half, bn_stats), combined in the group
reduction matmul with a half-weighted second accumulation; rstd uses a
Newton rsqrt on DVE int/float ALU so the only ACT functions ever used are
{Copy, Square, Identity, Exp} = one table set, preloaded during DMA wait.
"""

import ml_dtypes
import numpy as np

import concourse.bass as bass
import concourse.tile as tile
from concourse import bacc, mybir
from concourse import bass_utils

F32 = mybir.dt.float32
F32R = mybir.dt.float32r
BF16 = mybir.dt.bfloat16
FP8 = mybir.dt.float8e4
I32 = mybir.dt.int32
DR = mybir.MatmulPerfMode.DoubleRow
AOP = mybir.AluOpType

NPF8 = ml_dtypes.float8_e4m3fn

B, C, D, H, W = 2, 512, 4, 32, 32
L = D * H * W            # 4096
G = 32                   # groupnorm groups
EPS = 1e-6
P = 128
NT = C // P              # 4 channel tiles
NJ = L // P              # 32 key tiles
IC = 512                 # i-chunk width
LQ = 1024                # query cols per core
NIC = LQ // IC           # 2 i-chunks
NCORES = 8
NPAIR = NJ // 2          # 16 key-tile pairs per i-chunk
PD = 3                   # attention software-pipeline depth (es pairs ahead)

LH = 1536                # stats split: ACT does cols [0,LH), DVE does [LH,L)
DCH = [512, 512, 512, 512, 512]   # DVE bn_stats chunk widths (sum = L-LH)

QK_SCALE = 256.0         # host prescale on Wqk (fp8 subnormal avoidance)
OV_SCALE = 1.0           # no Wov prescale -> softmax denom needs no unfold
ES_SHIFT = -5.0          # softmax-invariant logit shift (max logit ~9.2)

_CACHE = {}


def _build():
    nc = bacc.Bacc(trn_type="TRN2", target_bir_lowering=False, debug=False,
                   num_devices=NCORES)
    x_d = nc.dram_tensor("x8", [NT, P, L], FP8, kind="ExternalInput").ap()
    xf_d = nc.dram_tensor("xf", [P, NT * LQ], BF16, kind="ExternalInput").ap()
    wqk_d = nc.dram_tensor("wqkT", [P, NT * C], FP8, kind="ExternalInput").ap()
    wov_d = nc.dram_tensor("wovT", [P, NT * C], FP8, kind="ExternalInput").ap()
    pg_d = nc.dram_tensor("pg", [C, G], F32, kind="ExternalInput").ap()
    pgh_d = nc.dram_tensor("pgh", [C, G], F32, kind="ExternalInput").ap()
    sel_d = nc.dram_tensor("sel", [G, C], F32R, kind="ExternalInput").ap()
    nwt_d = nc.dram_tensor("nwt", [G, 4], F32, kind="ExternalInput").ap()
    onesF_d = nc.dram_tensor("onesF", [P, P], F32R, kind="ExternalInput").ap()
    gamma_d = nc.dram_tensor("gamma", [C], F32, kind="ExternalInput").ap()
    wg_d = nc.dram_tensor("wgT", [G, C], F32R, kind="ExternalInput").ap()
    vg_d = nc.dram_tensor("vgT", [G, C], F32R, kind="ExternalInput").ap()
    hqk_d = nc.dram_tensor("hqk", [C], F32, kind="ExternalInput").ap()
    hov_d = nc.dram_tensor("hov", [C], F32, kind="ExternalInput").ap()
    out_d = nc.dram_tensor("out", [C, LQ], BF16, kind="ExternalOutput").ap()

    with tile.TileContext(nc) as tc:
        with (
            tc.tile_pool(name="big", bufs=1) as big,
            tc.tile_pool(name="wp", bufs=1) as wp,
            tc.tile_pool(name="small", bufs=1) as small,
            tc.tile_pool(name="est", bufs=PD + 4) as est,
            tc.tile_pool(name="osb", bufs=3) as osb,
            tc.tile_pool(name="obf", bufs=4) as obf,
            tc.tile_pool(name="accp", bufs=4) as accp,
            tc.tile_pool(name="zp", bufs=8) as zp,
            tc.tile_pool(name="tmp", bufs=4) as tmp,
            tc.tile_pool(name="ps", bufs=3, space="PSUM") as ps,
            tc.tile_pool(name="pho", bufs=4, space="PSUM") as pho,
            tc.tile_pool(name="psum1", bufs=1, space="PSUM") as psum1,
        ):
            # ---- DMA: x8 tiles split across the two HWDGE rings
            # (partition-major: 4KB rows per tile -> big packets) ----
            xt8 = big.tile([P, NT, L], FP8, tag="xt8")
            for t in (0, 1):
                nc.sync.dma_start(xt8[:, t, :], x_d[t])
            for t in (2, 3):
                nc.scalar.dma_start(xt8[:, t, :], x_d[t])
            wqk = wp.tile([P, NT, C], FP8, tag="wqk")
            nc.scalar.dma_start(wqk[:], wqk_d)
            wov = wp.tile([P, NT, C], FP8, tag="wov")
            nc.scalar.dma_start(wov[:], wov_d)
            xf = big.tile([P, NT, LQ], BF16, tag="xf")
            nc.sync.dma_start(xf[:], xf_d)
            pg = small.tile([P, NT, G], F32, tag="pg")
            nc.gpsimd.dma_start(pg[:], pg_d.rearrange("(t p) g -> p t g", p=P))
            pgh = small.tile([P, NT, G], F32, tag="pgh")
            nc.gpsimd.dma_start(pgh[:], pgh_d.rearrange("(t p) g -> p t g", p=P))
            sel = small.tile([G, NT, P], F32R, tag="sel")
            nc.gpsimd.dma_start(sel[:], sel_d.rearrange("g (t p) -> g t p", p=P))
            nwt = small.tile([G, 4], F32, tag="nwt")
            nc.gpsimd.dma_start(nwt[:], nwt_d)
            onesF = small.tile([P, P], F32R, tag="onesF")
            nc.gpsimd.dma_start(onesF[:], onesF_d)
            gam = small.tile([P, NT], F32, tag="gam")
            nc.gpsimd.dma_start(gam[:], gamma_d.rearrange("(t p) -> p t", p=P))
            wg = small.tile([G, NT, P], F32R, tag="wg")
            nc.gpsimd.dma_start(wg[:], wg_d.rearrange("g (t p) -> g t p", p=P))
            vg = small.tile([G, NT, P], F32R, tag="vg")
            nc.gpsimd.dma_start(vg[:], vg_d.rearrange("g (t p) -> g t p", p=P))
            hqk = small.tile([P, NT], F32, tag="hqk")
            nc.gpsimd.dma_start(hqk[:], hqk_d.rearrange("(t p) -> p t", p=P))
            hov = small.tile([P, NT], F32, tag="hov")
            nc.gpsimd.dma_start(hov[:], hov_d.rearrange("(t p) -> p t", p=P))

            # preload the exp table set (Copy/Square/Identity/Exp all live in
            # it -> this is the only ACT table load, during DMA wait)
            shf = small.tile([P, 1], F32, tag="shf")
            nc.vector.memset(shf[:], ES_SHIFT)

            epst = small.tile([G, 1], F32, tag="eps")
            nc.vector.memset(epst[:], EPS)
            dum = tmp.tile([P, 1], F32, tag="dum")
            nc.scalar.activation(dum[:], shf[:], mybir.ActivationFunctionType.Exp,
                                 bias=shf[:])

            # ---- groupnorm stats, per tile: ACT does cols [0,LH) via
            # Copy/Square accum (scaled 1/L), DVE does [LH,L) via bn_stats.
            # The halves meet in the group matmul: gps = pg.m2a + pgh.m2d
            # with pgh = pg * (L-LH)/L. ----
            m2a = small.tile([P, NT, 2], F32, tag="m2a")
            m2d = small.tile([P, NT, 2], F32, tag="m2d")
            ajunk = tmp.tile([P, LH], BF16, tag="ajunk")
            for t in (0, 2, 1, 3):
                nc.scalar.activation(ajunk[:], xt8[:, t, 0:LH],
                                     mybir.ActivationFunctionType.Copy,
                                     scale=1.0 / L,
                                     accum_out=m2a[:, t, 0:1])
                nc.scalar.activation(ajunk[:], xt8[:, t, 0:LH],
                                     mybir.ActivationFunctionType.Square,
                                     scale=1.0 / (L ** 0.5),
                                     accum_out=m2a[:, t, 1:2])
            for t in (0, 2, 1, 3):
                st = tmp.tile([P, len(DCH), 6], F32, tag="bnst")
                off = LH
                for s, w in enumerate(DCH):
                    nc.vector.bn_stats(st[:, s, :], xt8[:, t, off:off + w])
                    off += w
                mv = tmp.tile([P, 2], F32, tag="bnmv")
                nc.vector.bn_aggr(mv[:], st[:])
                # m2d = [mean_d, var_d + mean_d^2] = [mean_d, E_d[x^2]]
                msq = tmp.tile([P, 1], F32, tag="msq")
                nc.vector.tensor_mul(msq[:], mv[:, 0:1], mv[:, 0:1])
                nc.vector.tensor_copy(m2d[:, t, 0:1], mv[:, 0:1])
                nc.vector.tensor_add(m2d[:, t, 1:2], mv[:, 1:2], msq[:])
            gps = ps.tile([G, 2], F32, tag="mm")
            for t in range(NT):
                nc.tensor.matmul(gps[:], pg[:, t, :], m2a[:, t, :],
                                 start=(t == 0), stop=False)
            for t in range(NT):
                nc.tensor.matmul(gps[:], pgh[:, t, :], m2d[:, t, :],
                                 start=False, stop=(t == NT - 1))
            # group stats -> [mean_g, rstd_g]; rstd via Newton rsqrt on DVE
            gsb = small.tile([G, 2], F32R, tag="gsb")
            nc.vector.tensor_copy(gsb[:, 0:1], gps[:, 0:1])
            vrg = tmp.tile([G, 1], F32, tag="vrg")
            nc.vector.tensor_mul(vrg[:], gsb[:, 0:1].bitcast(F32), gsb[:, 0:1].bitcast(F32))
            nc.vector.tensor_tensor(vrg[:], gps[:, 1:2], vrg[:], AOP.subtract)
            nc.vector.tensor_tensor(vrg[:], vrg[:], epst[:], AOP.add)
            # y0 = bitcast(0x5f3759df - (bitcast_i32(v) >> 1)); 2 Newton steps
            magic = nwt[:, 0:1].bitcast(I32)
            one_i = nwt[:, 1:2].bitcast(I32)
            c15 = nwt[:, 2:3]
            ch = nwt[:, 3:4]
            yk = tmp.tile([G, 1], F32, tag="yk")
            nc.vector.tensor_tensor(yk[:].bitcast(I32), vrg[:].bitcast(I32),
                                    one_i, AOP.logical_shift_right)
            nc.vector.tensor_tensor(yk[:].bitcast(I32), magic,
                                    yk[:].bitcast(I32), AOP.subtract)
            vh = tmp.tile([G, 1], F32, tag="vh")
            nc.vector.tensor_tensor(vh[:], vrg[:], ch, AOP.mult)
            t1 = tmp.tile([G, 1], F32, tag="t1")
            for _ in range(1):
                nc.vector.tensor_mul(t1[:], yk[:], yk[:])
                nc.vector.tensor_mul(t1[:], t1[:], vh[:])
                nc.vector.tensor_tensor(t1[:], c15, t1[:], AOP.subtract)
                nc.vector.tensor_mul(yk[:], yk[:], t1[:])
            nc.vector.tensor_copy(gsb[:, 1:2], yk[:])
            # broadcast to channels: chsb[p, t, 0:2] = [mean, rstd] per channel
            chsb = small.tile([P, NT, 2], F32, tag="chsb")
            chs = ps.tile([P, 2 * NT], F32, tag="mm")
            for t in range(NT):
                nc.tensor.matmul(chs[:, 2 * t:2 * t + 2], sel[:, t, :], gsb[:],
                                 start=True, stop=True)
            nc.vector.tensor_copy(chsb[:], chs[:])

            # ---- quantize weights with the rstd fold (gamma is host-folded;
            # DVE so the PE unblocks without waiting on the ACT stats tail) ----
            wqk8 = wp.tile([P, NT, C], FP8, tag="wqk8")
            wov8 = wp.tile([P, NT, C], FP8, tag="wov8")
            for t in range(NT):
                nc.vector.tensor_tensor(wqk8[:, t, :], wqk[:, t, :],
                                        chsb[:, t, 1:2].to_broadcast((P, C)),
                                        AOP.mult)
                nc.scalar.activation(wov8[:, t, :], wov[:, t, :],
                                     mybir.ActivationFunctionType.Copy,
                                     scale=chsb[:, t, 1:2])
            # A = rstd*gamma per channel (cout-side fold for qk8)
            A = small.tile([P, NT], F32, tag="A")
            nc.vector.tensor_mul(A[:], chsb[:, :, 1], gam[:])
            As = small.tile([P, NT], F32, tag="As")
            nc.scalar.activation(As[:], A[:], mybir.ActivationFunctionType.Copy,
                                 scale=1.0 / QK_SCALE)

            # ---- qk8[:, i] = fp8(A.(WqkA x + bqkE)) over query cols.
            # The bias-fold matvecs are emitted after the first qps psum so
            # the PE starts the projection as soon as wqk8 lands; the biases
            # (DVE/PE smalls) complete well before the first qk8 ACT. ----
            qk8 = big.tile([P, NT, LQ], FP8, tag="qk8")
            bias_tiles = {}

            def emit_bias():
                st2 = small.tile([G, 2], F32R, tag="st2")
                nc.vector.tensor_mul(st2[:, 0:1], gsb[:, 0:1].bitcast(F32), gsb[:, 1:2].bitcast(F32))
                nc.vector.tensor_copy(st2[:, 1:2], gsb[:, 0:1].bitcast(F32))
                bqkE = small.tile([P, NT], F32, tag="bqkE")
                bovE = small.tile([P, NT], F32, tag="bovE")
                psB = ps.tile([P, 4 * NT], F32, tag="mm")
                for tq in range(NT):
                    nc.tensor.matmul(psB[:, 2 * tq:2 * tq + 2], wg[:, tq, :], st2[:],
                                     start=True, stop=True)
                    nc.tensor.matmul(psB[:, 2 * NT + 2 * tq:2 * NT + 2 * tq + 2],
                                     vg[:, tq, :], st2[:], start=True, stop=True)
                psBv = psB.rearrange("p (c two) -> p c two", two=2)
                nc.vector.tensor_tensor(bqkE[:], hqk[:], psBv[:, 0:NT, 0],
                                        AOP.subtract)
                nc.vector.tensor_tensor(bovE[:], hov[:], psBv[:, NT:2 * NT, 0],
                                        AOP.subtract)
                qkb = small.tile([P, NT], F32, tag="qkb")
                nc.vector.tensor_mul(qkb[:], A[:], bqkE[:])
                bias_tiles["bovE"] = bovE
                bias_tiles["qkb"] = qkb

            for icn in range(NIC):
                for tq in range(NT):
                    qps = ps.tile([P, IC], F32, tag="mm")
                    for u in range(2):
                        nc.tensor.matmul(qps[:], wqk8[:, 2 * u:2 * u + 2, bass.ts(tq, P)],
                                         xt8[:, 2 * u:2 * u + 2, bass.ts(icn, IC)],
                                         start=(u == 0), stop=(u == 1), perf_mode=DR)
                    if not bias_tiles:
                        emit_bias()
                    nc.scalar.activation(qk8[:, tq, bass.ts(icn, IC)], qps[:],
                                         mybir.ActivationFunctionType.Identity,
                                         scale=As[:, tq:tq + 1],
                                         bias=bias_tiles["qkb"][:, tq:tq + 1])

            # ---- z[t][icn] = x_residual + bovE on GpSimd (idle here) ----
            zall = {}
            for icn in range(NIC):
                for t in range(NT):
                    z = zp.tile([P, IC], F32, tag="zp", name=f"z{icn}_{t}")
                    nc.gpsimd.tensor_tensor(z[:], xf[:, t, bass.ts(icn, IC)],
                                            bias_tiles["bovE"][:, t:t + 1].to_broadcast((P, IC)),
                                            AOP.add)
                    zall[(icn, t)] = z

            # ---- voT projection: voT[j, c] = fp8((WovA x)[c, j]^T)
            # (psum->fp8 casts split DVE/ACT to keep pace with the PE) ----
            vot8 = big.tile([P, NJ, C], FP8, tag="vot8")
            for j in range(NJ):
                vps = ps.tile([P, C], F32, tag="mm")
                for u in range(2):
                    nc.tensor.matmul(vps[:], xt8[:, 2 * u:2 * u + 2, bass.ts(j, P)],
                                     wov8[:, 2 * u:2 * u + 2, :],
                                     start=(u == 0), stop=(u == 1), perf_mode=DR)
                if j % 2 == 0:
                    nc.vector.tensor_copy(vot8[:, j, :], vps[:])
                else:
                    nc.scalar.copy(vot8[:, j, :], vps[:])

            # ---- attention per i-chunk ----
            pending_fin = [None]

            def make_finalize(icn, sums, hops):
                def fin():
                    zs = [zall[(icn, t)] for t in range(NT)]
                    last = icn == NIC - 1
                    HW = IC // 2 if last else IC
                    rbc = tmp.tile([P, IC], BF16, tag="rbc", name=f"rbc{icn}")
                    for h in range(IC // HW):
                        hsl = slice(h * HW, (h + 1) * HW)
                        with nc.allow_low_precision(reason="softmax denom bf16"):
                            nc.vector.reciprocal(rbc[:, hsl], sums[:, hsl])
                        for t in range(NT):
                            o = osb.tile([P, HW], F32, tag="osb",
                                         name=f"o{icn}_{t}_{h}")
                            nc.vector.tensor_tensor(o[:], hops[t][:, hsl],
                                                    rbc[:, hsl], AOP.mult)
                            ob = obf.tile([P, HW], BF16, tag="obf",
                                          name=f"ob{icn}_{t}_{h}")
                            eng = nc.gpsimd if t >= 1 else nc.vector
                            eng.tensor_tensor(ob[:], o[:], zs[t][:, hsl],
                                              AOP.add)
                            deng = nc.sync if t < 2 else nc.scalar
                            deng.dma_start(
                                out_d[bass.ts(t, P),
                                      icn * IC + h * HW:icn * IC + (h + 1) * HW],
                                ob[:])
                return fin

            for icn in range(NIC):
                sums = psum1.tile([P, IC], F32, tag="sums", name=f"sums{icn}")
                hops = [pho.tile([P, IC], F32, tag="ho", name=f"ho_{icn}_{t}")
                        for t in range(NT)]
                # softmax denominator: es accumulated off the PE (DVE takes
                # even j, GpSimd odd j), partition-reduced by one f32r
                # matmul pair at the end of the chunk
                accv = accp.tile([P, IC], F32R, tag="accv", name=f"accv{icn}")
                accg = accp.tile([P, IC], F32R, tag="accg", name=f"accg{icn}")
                espairs = [None] * NPAIR

                def consume(u, hops=hops, espairs=espairs):
                    es = espairs[u]
                    for t in range(NT):
                        nc.tensor.matmul(hops[t][:],
                                         vot8[:, 2 * u:2 * u + 2, bass.ts(t, P)],
                                         es[:, 0:2, :],
                                         start=(u == 0), stop=(u == NPAIR - 1),
                                         perf_mode=DR)
                    espairs[u] = None

                escur = [None]
                for j in range(NJ):
                    u, par = divmod(j, 2)
                    if j == 2 and pending_fin[0] is not None:
                        pending_fin[0]()
                        pending_fin[0] = None
                    sps = ps.tile([P, IC], F32, tag="mm", name=f"sps{icn}_{j}")
                    for uu in range(2):
                        nc.tensor.matmul(sps[:], xt8[:, 2 * uu:2 * uu + 2, bass.ts(j, P)],
                                         qk8[:, 2 * uu:2 * uu + 2, bass.ts(icn, IC)],
                                         start=(uu == 0), stop=(uu == 1), perf_mode=DR)
                    if par == 0:
                        escur[0] = est.tile([P, 2, IC], FP8, tag="est",
                                            name=f"es{icn}_{u}")
                    nc.scalar.activation(escur[0][:, par, :], sps[:],
                                         mybir.ActivationFunctionType.Exp,
                                         bias=shf[:])
                    eng, acc = (nc.vector, accv) if par == 0 else (nc.gpsimd, accg)
                    if j < 2:
                        eng.tensor_copy(acc[:], escur[0][:, par, :])
                    else:
                        eng.tensor_tensor(acc[:], acc[:], escur[0][:, par, :],
                                          AOP.add)
                    if par == 1:
                        espairs[u] = escur[0]
                        if u >= PD:
                            consume(u - PD)
                for u in range(NPAIR - PD, NPAIR):
                    consume(u)
                nc.tensor.matmul(sums[:], onesF[:], accv[:], start=True,
                                 stop=False)
                nc.tensor.matmul(sums[:], onesF[:], accg[:], start=False,
                                 stop=True)
                pending_fin[0] = make_finalize(icn, sums, hops)
            pending_fin[0]()

    nc.compile()
    return nc


def _prep(inputs):
    s = float(C) ** -0.5
    wq = np.asarray(inputs["wq"], np.float64)
    wk = np.asarray(inputs["wk"], np.float64)
    wv = np.asarray(inputs["wv"], np.float64)
    wo = np.asarray(inputs["wo"], np.float64)
    bq = np.asarray(inputs["bq"], np.float64)
    bv = np.asarray(inputs["bv"], np.float64)
    bo = np.asarray(inputs["bo"], np.float64)
    gamma = np.asarray(inputs["gamma"], np.float64)
    beta = np.asarray(inputs["beta"], np.float64)
    Wqk = (wk.T @ wq).T * s      # lhsT layout [c_in, c_out]
    Wov = (wo @ wv).T            # [c_in, c_out]
    bqkv = (wk.T @ bq) * s
    bovv = wo @ bv + bo
    GS = C // G
    WgT = (Wqk * gamma[:, None]).reshape(G, GS, C).sum(axis=1)
    VgT = (Wov * gamma[:, None]).reshape(G, GS, C).sum(axis=1)
    pg = ((np.arange(C)[:, None] // GS == np.arange(G)[None, :])
          .astype(np.float32) / GS)
    dve_frac = 1.0 - LH / L
    # partition-major weight layouts: [p, t, c] flattened to [P, NT*C]
    wqkb = np.clip(Wqk * gamma[:, None] * QK_SCALE, -448, 448).astype(NPF8)
    wovb = np.clip(Wov * gamma[:, None] * OV_SCALE, -448, 448).astype(NPF8)
    nwt = np.zeros(4, np.float32)
    nwt_u = nwt.view(np.uint32)
    nwt_u[0] = 0x5F3759DF
    nwt_u[1] = 1
    nwt[2] = 1.5
    nwt[3] = 0.5
    consts = {
        "wqkT": np.ascontiguousarray(
            wqkb.reshape(NT, P, C).transpose(1, 0, 2).reshape(P, NT * C)),
        "wovT": np.ascontiguousarray(
            wovb.reshape(NT, P, C).transpose(1, 0, 2).reshape(P, NT * C)),
        "wgT": np.ascontiguousarray(WgT, np.float32),
        "vgT": np.ascontiguousarray(VgT, np.float32),
        "hqk": (Wqk.T @ beta + bqkv).astype(np.float32),
        "hov": (Wov.T @ beta + bovv).astype(np.float32),
        "gamma": np.asarray(inputs["gamma"], np.float32),
        "pg": np.ascontiguousarray(pg),
        "pgh": np.ascontiguousarray(pg * dve_frac),
        "sel": np.ascontiguousarray(
            (np.arange(G)[:, None] == np.arange(C)[None, :] // GS)
            .astype(np.float32)),
        "nwt": np.ascontiguousarray(np.tile(nwt.reshape(1, 4), (G, 1))),
        "onesF": np.ones((P, P), np.float32),
    }
    return consts


LAST_RESULTS = None


def kernel(**inputs) -> np.ndarray:
    global LAST_RESULTS
    if "nc" not in _CACHE:
        _CACHE["nc"] = _build()
    nc = _CACHE["nc"]
    consts = _prep(inputs)
    x = np.asarray(inputs["x"], np.float32)
    xb = x.reshape(B, C, L)
    in_maps = []
    for core in range(NCORES):
        b, chunk = divmod(core, 4)
        xr = np.roll(xb[b], -LQ * chunk, axis=1)
        # x8: [t][p][l] partition-major per tile (4KB DRAM rows)
        x8 = np.ascontiguousarray(xr.reshape(NT, P, L)).astype(NPF8)
        # xf: [p][t*LQ] partition-major (16KB rows)
        xf = np.ascontiguousarray(
            xr[:, :LQ].reshape(NT, P, LQ).transpose(1, 0, 2)
            .reshape(P, NT * LQ)).astype(ml_dtypes.bfloat16)
        in_maps.append({"x8": x8, "xf": xf, **consts})
    res = bass_utils.run_bass_kernel_spmd(nc, in_maps, core_ids=list(range(NCORES)))
    LAST_RESULTS = res
    out = np.empty((B, C, L), np.float32)
    for core in range(NCORES):
        b, chunk = divmod(core, 4)
        out[b][:, LQ * chunk:LQ * (chunk + 1)] = \
            np.asarray(res.results[core]["out"], np.float32)
    return out.reshape(B, C, D, H, W)
